# revision 1
# baseline (speedup 1.0000x reference)
"""LIF spike recurrence kernel for Trainium2 (8 NeuronCores, SPMD).

Problem: x [32, 128, 32, 32, 8] f32, recurrence over last (time) dim:
    u_t = TAU * u_{t-1} * (1 - o_{t-1}) + x_t
    o_t = 1[u_t - VTH > 0]
Output: o [32, 128, 32, 32, 8] f32 (0.0 / 1.0 spikes).

Strategy:
  - Shard batch dim (32) across 8 cores -> 4 per core. Pure elementwise over
    pixels; no communication.
  - Per core: 524288 pixels x 8 timesteps, viewed as DRAM [128, 4096, 8]
    (partition-major, each partition one contiguous 128KB run).
  - Tiles [128, F, 8]. Recurrence runs IN PLACE in the interleaved tile:
    membrane u_t overwrites x_t slice via two fused scalar_tensor_tensor ops
    per step; a single contiguous 2x-mode tensor_scalar pass converts the
    whole tile of membrane potentials into spikes at the end.
  - Exact fp32 equivalence with the reference:
      mask = (u <= VTH) in {0.0, 1.0};  c = mask*u;  u' = c*0.25 + x_t
    gives bitwise the same values as TAU*u*(1-o)+x_t (mult by 0/1 and by
    2^-2 are exact), and (u > VTH) == (u - VTH > 0) in fp32 because the
    rounded difference never flips sign (Sterbenz + magnitude arguments).
"""

import numpy as np

TAU = 0.25
VTH = 0.3
N_CORES = 8
P = 128
T = 8
B_LOC = 4  # batches per core
PIX_PER_CORE = B_LOC * 128 * 32 * 32  # 524288
NPP = PIX_PER_CORE // P  # 4096 pixels per partition
F = 1024  # pixels per partition per tile
N_TILES = NPP // F

_CACHE = {}


def _build_nc():
    import concourse.tile as tile
    from concourse import bacc, mybir

    Alu = mybir.AluOpType
    f32 = mybir.dt.float32

    nc = bacc.Bacc(
        "TRN2",
        target_bir_lowering=False,
        debug=False,
        enable_asserts=False,
        num_devices=N_CORES,
    )
    x_d = nc.dram_tensor("x", [P, NPP, T], f32, kind="ExternalInput").ap()
    o_d = nc.dram_tensor("o", [P, NPP, T], f32, kind="ExternalOutput").ap()

    with tile.TileContext(nc) as tc:
        with tc.tile_pool(name="xp", bufs=3) as xp, tc.tile_pool(
            name="cp", bufs=2
        ) as cp:
            for i in range(N_TILES):
                xt = xp.tile([P, F, T], f32, tag="xt")
                nc.sync.dma_start(xt[:], x_d[:, i * F : (i + 1) * F, :])
                # u_0 = x_0 is already in place at slice 0.
                for t in range(1, T):
                    up = xt[:, :, t - 1]
                    c = cp.tile([P, F], f32, tag="c")
                    # c = (u_prev <= VTH) * u_prev
                    nc.vector.scalar_tensor_tensor(
                        c[:], up, VTH, up, op0=Alu.is_le, op1=Alu.mult
                    )
                    # u_t = c * TAU + x_t   (in place over x_t slice)
                    nc.vector.scalar_tensor_tensor(
                        xt[:, :, t], c[:], TAU, xt[:, :, t], op0=Alu.mult, op1=Alu.add
                    )
                # Whole-tile spike threshold, contiguous, in place.
                flat = xt.rearrange("p f t -> p (f t)")
                nc.vector.tensor_scalar(
                    flat, flat, VTH, None, op0=Alu.is_gt
                )
                nc.sync.dma_start(o_d[:, i * F : (i + 1) * F, :], xt[:])
    nc.compile()
    return nc


def _get_nc():
    if "nc" not in _CACHE:
        _CACHE["nc"] = _build_nc()
    return _CACHE["nc"]


def _shard(x: np.ndarray):
    xs = np.ascontiguousarray(x, dtype=np.float32)
    return [
        np.ascontiguousarray(xs[i * B_LOC : (i + 1) * B_LOC].reshape(P, NPP, T))
        for i in range(N_CORES)
    ]


def _run(in_maps, **kwargs):
    from concourse.bass_utils import run_bass_kernel_spmd

    nc = _get_nc()
    return run_bass_kernel_spmd(nc, in_maps, core_ids=list(range(N_CORES)), **kwargs)


def kernel(x: np.ndarray) -> np.ndarray:
    in_maps = [{"x": s} for s in _shard(x)]
    res = _run(in_maps)
    outs = [
        res.results[i]["o"].reshape(B_LOC, 128, 32, 32, T) for i in range(N_CORES)
    ]
    return np.concatenate(outs, axis=0)
